# revision 48
# baseline (speedup 1.0000x reference)
"""EulerCE attention Trainium2 kernel (bf16, interleaved schedule).

Sharding: data-parallel over batch (2) x head-parallel over 4 head-groups
(16 heads / 4 per group) = 8 cores. Core c: batch c//4, heads 4*(c%4)..+4.

Per-core math (head group g, batch b):
  - All matmul operands in bf16 (fast weight loads); PSUM stays f32.
  - Full xT staged to SBUF up-front, DMA order chosen so the first
    projection's operands land first (PE starts ~6us in).
  - QKV projection with host-permuted weight rows so Q/K come out in
    "stacked evens/odds" layout ready for a full-128-partition RoPE-style
    rotation on DVE; V computed in [n, dh] orientation directly.
  - scores computed transposed: s^T[k, q] = K_h^T-slice . Q_h-slice,
    decay bias folded into the exp's per-partition bias (c_h * k is a
    per-partition value in this layout; the -c_h*q per-row term cancels in
    softmax), causal mask applied only on exact-diagonal 128x128 subtiles
    (Pool engine, SBUF-only).
  - softmax without max-subtraction (scores provably small for this data),
    denominator obtained by 64 ones-columns in the PV stationary operand
    which makes the PE replicate sum_k P across 64 partitions for free.
  - softmax normalization uses reciprocal_approx_fast after a tracked
    PSUM->SBUF copy (custom-DVE ops don't get cross-engine dep edges).
  - phases are split into ~1-2us issue units and interleaved: attention
    tiles run between projection/O-projection units so the PE always has
    independent work while exp/normalize chains drain, and stays dense
    enough that HAM keeps the PE at full clock.
  - PSUM (8 banks): score tiles 4x1 (two tiles/attention-tile, 2-deep
    lookahead), PV accumulators 4x1 (pairs fully double-buffered),
    projection pair-tiles 2x2-bank ring shared with the V-proj psum and
    the O-proj outputs.
  - O-projection consumes attn^T directly; per-core partial outputs (bf16)
    are summed on host across the 4 head-group cores of each batch.
"""

import sys

sys.path.insert(0, "/opt/trn_rl_repo")

import math

import ml_dtypes
import numpy as np

import concourse.bass as bass
from concourse import bacc
import concourse.mybir as mybir
import concourse.tile as tile
from concourse.bass_utils import run_bass_kernel_spmd

F32 = mybir.dt.float32
BF16 = mybir.dt.bfloat16
EXP = mybir.ActivationFunctionType.Exp
NP_BF16 = ml_dtypes.bfloat16

D_MODEL = 1024
N_HEADS = 16
D_HEAD = 64
BATCH = 2
SEQ = 2048
H_LOC = 4          # heads per core
CH = 512           # n-chunk (= strip) size
NCH = SEQ // CH    # 4 chunks
KT = 128           # k tile
NT = SEQ // KT     # 16 n-tiles


def build_program(reps=1):
    nc = bacc.Bacc()
    xT = nc.dram_tensor("xT", [D_MODEL, SEQ], BF16, kind="ExternalInput")
    wqk = nc.dram_tensor("wqk", [D_MODEL, 512], BF16, kind="ExternalInput")
    wv = nc.dram_tensor("wv", [D_MODEL, 256], BF16, kind="ExternalInput")
    wo = nc.dram_tensor("wo", [256, D_MODEL], BF16, kind="ExternalInput")
    cost = nc.dram_tensor("cost", [128, SEQ], F32, kind="ExternalInput")
    sint = nc.dram_tensor("sint", [128, SEQ], F32, kind="ExternalInput")
    biast = nc.dram_tensor("biast", [128, H_LOC * NT], F32, kind="ExternalInput")
    maskt = nc.dram_tensor("maskt", [128, 128], BF16, kind="ExternalInput")
    out = nc.dram_tensor("out", [SEQ, D_MODEL], BF16, kind="ExternalOutput")

    with tile.TileContext(nc) as tc:
        with (
            tc.tile_pool(name="consts", bufs=1) as consts,
            tc.tile_pool(name="persist", bufs=1) as persist,
            tc.tile_pool(name="rot", bufs=2) as rotp,
            tc.tile_pool(name="ptp", bufs=6) as ptp,
            tc.tile_pool(name="attnp", bufs=2) as attnp,
            tc.tile_pool(name="recp", bufs=2) as recp,
            tc.tile_pool(name="obp", bufs=2) as obp,
            tc.tile_pool(name="spps", bufs=4, space="PSUM") as spps,
            tc.tile_pool(name="prps", bufs=2, space="PSUM") as prps,
            tc.tile_pool(name="avps", bufs=2, space="PSUM") as avps,
        ):
            # ---- inputs staged in, ordered so the first projection's
            # operands (x chunk 0, wqk, wv) land first
            x_sb = persist.tile([128, 8, SEQ], BF16, tag="xsb")
            wqk_sb = consts.tile([128, 8, 512], BF16, tag="wqk")
            # first-half (k 0:4) operands of both x and wqk land first so the
            # k<4 accumulation steps of the first m-block start ~3us earlier
            nc.sync.dma_start(
                out=x_sb[:, 0:4, 0:CH],
                in_=xT[0:512, 0:CH].rearrange("(k p) m -> p k m", p=128),
            )
            nc.sync.dma_start(
                out=wqk_sb[:, 0:4, :],
                in_=wqk[0:512, :].rearrange("(k p) m -> p k m", p=128),
            )
            nc.sync.dma_start(
                out=x_sb[:, 4:8, 0:CH],
                in_=xT[512:1024, 0:CH].rearrange("(k p) m -> p k m", p=128),
            )
            nc.sync.dma_start(
                out=wqk_sb[:, 4:8, :],
                in_=wqk[512:1024, :].rearrange("(k p) m -> p k m", p=128),
            )
            wv_sb = consts.tile([128, 8, 256], BF16, tag="wv")
            nc.sync.dma_start(out=wv_sb, in_=wv.rearrange("(k p) m -> p k m", p=128))
            bias_sb = consts.tile([128, H_LOC * NT], F32, tag="bias")
            nc.sync.dma_start(out=bias_sb, in_=biast[:, :])
            mask_sb = consts.tile([128, 128], BF16, tag="mask")
            nc.sync.dma_start(out=mask_sb, in_=maskt[:, :])
            cos_sb = consts.tile([128, SEQ], F32, tag="cos")
            sin_sb = consts.tile([128, SEQ], F32, tag="sin")
            nc.sync.dma_start(out=cos_sb[:, 0:CH], in_=cost[:, 0:CH])
            nc.sync.dma_start(out=sin_sb[:, 0:CH], in_=sint[:, 0:CH])
            for c in range(1, NCH):
                nc.sync.dma_start(
                    out=x_sb[:, :, c * CH:(c + 1) * CH],
                    in_=xT[:, c * CH:(c + 1) * CH].rearrange(
                        "(k p) m -> p k m", p=128),
                )
                nc.sync.dma_start(out=cos_sb[:, c * CH:(c + 1) * CH],
                                  in_=cost[:, c * CH:(c + 1) * CH])
                nc.sync.dma_start(out=sin_sb[:, c * CH:(c + 1) * CH],
                                  in_=sint[:, c * CH:(c + 1) * CH])
            wo_sb = consts.tile([128, 2, D_MODEL], BF16, tag="wo")
            nc.sync.dma_start(out=wo_sb, in_=wo.rearrange("(k p) m -> p k m", p=128))

            # V in [n, dh] layout: [128, ntile, head, 128]; per head block,
            # cols 0:64 = V, cols 64:128 = ones (denominator-replication
            # trick). memset through an f32 view: two bf16 1.0s per f32 lane
            # (native bf16 memset patterns are unreliable on HW)
            # memset on the otherwise-idle Pool engine: at kernel start the
            # DVE queue is the critical path (rotations -> repack -> scores)
            v_sb = persist.tile([128, NT, H_LOC, 128], BF16, tag="vsb")
            ones2 = float(np.frombuffer(np.uint32(0x3F803F80).tobytes(), np.float32)[0])
            nc.gpsimd.memset(v_sb.bitcast(F32)[:, :, :, 32:64], ones2)

            # packed rotated Q/K, head-pair layout
            qb = [persist.tile([128, SEQ], BF16, tag=f"qb{j}", name=f"qb{j}") for j in range(2)]
            kb = [persist.tile([128, SEQ], BF16, tag=f"kb{j}", name=f"kb{j}") for j in range(2)]

            attn_tiles = {}  # (strip, pair) -> sbuf tile [128, 512] bf16

            def rotate(pe, po, dst, c0):
                # pe/po: psum [128, CH] stacked evens/odds for 4 heads
                # dst: [buf01, buf23]; writes rotated head-pair-packed layout
                t1 = rotp.tile([128, CH], F32, tag="t1")
                t2 = rotp.tile([128, CH], F32, tag="t2")
                top = rotp.tile([128, CH], BF16, tag="top")
                bot = rotp.tile([128, CH], BF16, tag="bot")
                cs = cos_sb[:, c0:c0 + CH]
                sn = sin_sb[:, c0:c0 + CH]
                nc.vector.tensor_mul(t1[:, :], pe[:, :], cs)
                nc.vector.tensor_mul(t2[:, :], po[:, :], sn)
                nc.vector.tensor_sub(top[:, :], t1[:, :], t2[:, :])
                nc.vector.tensor_mul(t1[:, :], pe[:, :], sn)
                nc.vector.tensor_mul(t2[:, :], po[:, :], cs)
                nc.vector.tensor_add(bot[:, :], t1[:, :], t2[:, :])
                # repack: head h (32-row group) -> buf h//2, rows 64*(h%2)+{0:32 top, 32:64 bot}
                for h in range(4):
                    b = dst[h // 2]
                    r0 = 64 * (h % 2)
                    nc.sync.dma_start(out=b[r0:r0 + 32, c0:c0 + CH], in_=top[32 * h:32 * h + 32, :])
                    nc.sync.dma_start(out=b[r0 + 32:r0 + 64, c0:c0 + CH], in_=bot[32 * h:32 * h + 32, :])

            def qk_units(c):
                # generator: yields after each ~1-2us PE unit so the driver
                # can interleave attention tiles between units. Rotations
                # reach the DVE queue right after each pair block -- the
                # next strip's scores wait on the repack.
                c0 = c * CH
                for mp in range(2):
                    pe = prps.tile([128, CH], F32, tag="pr", name="pe")
                    po = prps.tile([128, CH], F32, tag="pr", name="po")
                    for m2, dst in ((0, pe), (1, po)):
                        m = 2 * mp + m2
                        for k in range(8):
                            nc.tensor.matmul(
                                dst[:, :],
                                wqk_sb[:, k, m * 128:(m + 1) * 128],
                                x_sb[:, k, c0:c0 + CH],
                                start=(k == 0), stop=(k == 7),
                            )
                        yield
                    rotate(pe, po, qb if mp == 0 else kb, c0)

            def v_units(c):
                # chunks 0-1: psum->v_sb copies on the Scalar engine (ACT is
                # ~72% idle before the first strips, while the DVE queue --
                # rotations ahead of these copies -- gates the first PVs).
                # Later chunks stay on DVE: ACT is exp-saturated by then.
                c0 = c * CH
                for it in range(4):
                    t = 4 * c + it
                    vp = prps.tile([128, 256], F32, tag="pr", name="vp")
                    for k in range(8):
                        nc.tensor.matmul(
                            vp[:, :],
                            x_sb[:, k, c0 + it * 128:c0 + (it + 1) * 128],
                            wv_sb[:, k, :],
                            start=(k == 0), stop=(k == 7),
                        )
                    src = vp[:, :].rearrange("p (h d) -> p h d", h=4)
                    if c < 2:
                        nc.scalar.copy(out=v_sb[:, t, :, 0:64], in_=src)
                    else:
                        nc.vector.tensor_copy(out=v_sb[:, t, :, 0:64], in_=src)
                    yield

            def proj_units(c):
                yield from qk_units(c)
                yield from v_units(c)

            def attn_units(s):
                q0 = s * CH
                ntile_hi = 4 * s + 4
                for pair in range(2):
                    avs = [
                        avps.tile([128, CH], F32, tag="av", name=f"av_{s}_{pair}_{hl}")
                        for hl in range(2)
                    ]

                    def tile_geom(t):
                        r = t - 4 * s
                        qoff = 128 * r if r >= 0 else 0
                        return qoff, CH - qoff, r

                    def issue_scores(t):
                        qoff, w, r = tile_geom(t)
                        pts = []
                        for hl in range(2):
                            h = pair * 2 + hl
                            r0 = 64 * hl
                            sp = spps.tile([128, CH], F32, tag="sp", name=f"sp{hl}")
                            nc.tensor.matmul(
                                sp[:, 0:w],
                                kb[pair][r0:r0 + 64, t * KT:(t + 1) * KT],
                                qb[pair][r0:r0 + 64, q0 + qoff:q0 + CH],
                                start=True, stop=True,
                            )
                            pt = ptp.tile([128, CH], BF16, tag="pt", name=f"pt{hl}")
                            col = h * NT + t
                            nc.scalar.activation(
                                out=pt[:, 0:w], in_=sp[:, 0:w], func=EXP,
                                bias=bias_sb[:, col:col + 1], scale=1.0,
                            )
                            if r >= 0:
                                nc.gpsimd.tensor_mul(pt[:, 0:128], pt[:, 0:128], mask_sb[:, :])
                            pts.append(pt)
                        return pts

                    def issue_pv(t, pts, first=False, last=False):
                        qoff, w, r = tile_geom(t)
                        for hl in range(2):
                            h = pair * 2 + hl
                            nc.tensor.matmul(
                                avs[hl][:, qoff:CH],
                                v_sb[:, t, h, :],
                                pts[hl][:, 0:w],
                                start=first, stop=last,
                            )

                    # software pipeline: scores one tile ahead of PV
                    prev = None
                    for idx, t in enumerate(range(ntile_hi)):
                        cur = (t, issue_scores(t))
                        if prev is not None:
                            issue_pv(prev[0], prev[1], first=(idx == 1))
                        prev = cur
                        yield
                    issue_pv(prev[0], prev[1], last=True)

                    for hl in range(2):
                        r0 = 64 * hl
                        rec = recp.tile([64, CH], F32, tag="rec")
                        # tracked copy PSUM->SBUF first: custom-DVE ops don't
                        # get cross-engine dependency edges from Tile, so the
                        # copy provides the PE->DVE semaphore; the in-place
                        # approx then runs same-engine in-order
                        nc.vector.tensor_copy(out=rec[:, :], in_=avs[hl][64:128, :])
                        nc.vector.reciprocal_approx_fast(out=rec[:, :], in_=rec[:, :])
                        at = attn_tiles.get((s, pair))
                        if at is None:
                            at = attnp.tile([128, CH], BF16, tag=f"attn{pair}", name=f"attn_{s}_{pair}")
                            attn_tiles[(s, pair)] = at
                        nc.vector.tensor_mul(at[r0:r0 + 64, :], avs[hl][0:64, :], rec[:, :])
                    yield

            def oproj_units(s, copy_engine=None, fine=False):
                # copy_engine: the final strip runs its PSUM->SBUF copies on
                # the by-then-idle ACT engine so the busy DVE queue doesn't
                # delay the output tail. fine=True yields per half-tile for
                # denser interleaving.
                for it in range(4):
                    i = 4 * s + it
                    ob = obp.tile([128, 2, CH], BF16, tag="ob", name="ob")
                    for half in range(2):
                        op = spps.tile([128, CH], F32, tag="sp", name="op")
                        for ks in range(2):
                            nc.tensor.matmul(
                                op[:, :],
                                attn_tiles[(s, ks)][:, it * 128:(it + 1) * 128],
                                wo_sb[:, ks, half * CH:(half + 1) * CH],
                                start=(ks == 0), stop=(ks == 1),
                            )
                        if copy_engine == "scalar":
                            nc.scalar.copy(out=ob[:, half, :], in_=op[:, :])
                        else:
                            nc.vector.tensor_copy(out=ob[:, half, :], in_=op[:, :])
                        nc.sync.dma_start(
                            out=out[i * 128:(i + 1) * 128, half * CH:(half + 1) * CH],
                            in_=ob[:, half, :],
                        )
                        if fine:
                            yield
                    if not fine:
                        yield

            _END = object()

            def drain(g):
                for _ in g:
                    pass

            def interleave(fg, bg, ratio=1):
                # issue `ratio` foreground units, then 1 background unit,
                # until the foreground is exhausted; bg is NOT drained
                while True:
                    for _ in range(ratio):
                        if next(fg, _END) is _END:
                            return
                    next(bg, _END)

            def chain(*gens):
                for g in gens:
                    yield from g

            # attention strip s only needs chunks <= s. Strips interleave
            # at the tile level with the next chunk's projection and the
            # previous strip's O-projection: the PE always has independent
            # background work while the exp/normalize chains drain.
            for _rep in range(reps):
                attn_tiles.clear()
                drain(proj_units(0))
                bg = chain(proj_units(1))
                interleave(attn_units(0), bg)
                drain(bg)
                bg = chain(proj_units(2), oproj_units(0))
                interleave(attn_units(1), bg)
                drain(bg)
                bg = chain(qk_units(3), oproj_units(1))
                interleave(attn_units(2), bg, ratio=3)
                drain(bg)
                bg = chain(v_units(3), oproj_units(2, fine=True))
                interleave(attn_units(3), bg, ratio=3)
                drain(bg)
                drain(oproj_units(3, copy_engine="scalar"))

    return nc


def _sigmoid(v):
    return 1.0 / (1.0 + np.exp(-v.astype(np.float64)))


def build_inputs(x, Wqkv, Wo, log_xi, pi_gate_logit, e_gate_logit):
    x = np.asarray(x, np.float32)
    Wqkv = np.asarray(Wqkv, np.float32)
    Wo = np.asarray(Wo, np.float32)
    log_xi = np.asarray(log_xi, np.float32)
    pi_gate_logit = np.asarray(pi_gate_logit, np.float32)
    e_gate_logit = np.asarray(e_gate_logit, np.float32)

    pi_g = _sigmoid(pi_gate_logit)                      # (16,)
    c_h = (_sigmoid(e_gate_logit) / np.exp(log_xi.astype(np.float64)))  # (16,)

    Wq = Wqkv[0:1024].reshape(N_HEADS, D_HEAD, D_MODEL)
    Wk = Wqkv[1024:2048].reshape(N_HEADS, D_HEAD, D_MODEL)
    Wv = Wqkv[2048:3072].reshape(N_HEADS, D_HEAD, D_MODEL)

    f = np.arange(32)
    inv_freq = np.float64(math.pi) ** (1.0 - 2.0 * f / 64.0)            # (32,)
    pos = np.arange(SEQ, dtype=np.float64)

    mask01 = (np.arange(128)[:, None] <= np.arange(128)[None, :]).astype(NP_BF16)

    in_maps = []
    xTb = [np.ascontiguousarray(x[b].T).astype(NP_BF16) for b in range(BATCH)]
    for core in range(8):
        b, g = core // 4, core % 4
        hs = slice(4 * g, 4 * g + 4)
        qe = (Wq[hs, 0::2, :] * 0.125).reshape(128, D_MODEL)
        qo = (Wq[hs, 1::2, :] * 0.125).reshape(128, D_MODEL)
        ke = Wk[hs, 0::2, :].reshape(128, D_MODEL)
        ko = Wk[hs, 1::2, :].reshape(128, D_MODEL)
        wqk = np.ascontiguousarray(np.concatenate([qe, qo, ke, ko], 0).T).astype(NP_BF16)
        wv = np.ascontiguousarray(Wv[hs].reshape(256, D_MODEL).T).astype(NP_BF16)
        wo = np.ascontiguousarray(Wo[:, 256 * g:256 * (g + 1)].T).astype(NP_BF16)

        theta = pos[None, None, :] * inv_freq[None, :, None] * pi_g[4 * g:4 * g + 4, None, None]
        cost = np.cos(theta).reshape(128, SEQ).astype(np.float32)
        sint = np.sin(theta).reshape(128, SEQ).astype(np.float32)

        biast = np.empty((128, H_LOC * NT), np.float32)
        p = np.arange(128, dtype=np.float64)
        for hl in range(H_LOC):
            for t in range(NT):
                biast[:, hl * NT + t] = (c_h[4 * g + hl] * (128 * t + p)).astype(np.float32)

        in_maps.append({
            "xT": xTb[b], "wqk": wqk, "wv": wv, "wo": wo,
            "cost": cost, "sint": sint, "biast": biast, "maskt": mask01,
        })
    return in_maps


def kernel(x, Wqkv, Wo, log_xi, pi_gate_logit, e_gate_logit):
    in_maps = build_inputs(x, Wqkv, Wo, log_xi, pi_gate_logit, e_gate_logit)
    nc = build_program()
    nc.finalize()
    res = run_bass_kernel_spmd(nc, in_maps, list(range(8))).results
    out = np.zeros((BATCH, SEQ, D_MODEL), np.float32)
    for core in range(8):
        out[core // 4] += np.asarray(res[core]["out"]).astype(np.float32)
    return out


# revision 50
# speedup vs baseline: 1.0333x; 1.0333x over previous
"""EulerCE attention Trainium2 kernel (bf16, interleaved schedule).

Sharding: data-parallel over batch (2) x head-parallel over 4 head-groups
(16 heads / 4 per group) = 8 cores. Core c: batch c//4, heads 4*(c%4)..+4.

Per-core math (head group g, batch b):
  - All matmul operands in bf16 (fast weight loads); PSUM stays f32.
  - Full xT staged to SBUF up-front, DMA order chosen so the first
    projection's operands land first (PE starts ~6us in).
  - QKV projection with host-permuted weight rows so Q/K come out in
    "stacked evens/odds" layout ready for a full-128-partition RoPE-style
    rotation on DVE; V computed in [n, dh] orientation directly.
  - scores computed transposed: s^T[k, q] = K_h^T-slice . Q_h-slice,
    decay bias folded into the exp's per-partition bias (c_h * k is a
    per-partition value in this layout; the -c_h*q per-row term cancels in
    softmax), causal mask applied only on exact-diagonal 128x128 subtiles
    (Pool engine, SBUF-only).
  - softmax without max-subtraction (scores provably small for this data),
    denominator obtained by 64 ones-columns in the PV stationary operand
    which makes the PE replicate sum_k P across 64 partitions for free.
  - softmax normalization uses reciprocal_approx_fast after a tracked
    PSUM->SBUF copy (custom-DVE ops don't get cross-engine dep edges).
  - phases are split into ~1-2us issue units and interleaved: attention
    tiles run between projection/O-projection units so the PE always has
    independent work while exp/normalize chains drain, and stays dense
    enough that HAM keeps the PE at full clock.
  - PSUM (8 banks): score tiles 4x1 (two tiles/attention-tile, 2-deep
    lookahead), PV accumulators 4x1 (pairs fully double-buffered),
    projection pair-tiles 2x2-bank ring shared with the V-proj psum and
    the O-proj outputs.
  - O-projection consumes attn^T directly; per-core partial outputs (bf16)
    are summed on host across the 4 head-group cores of each batch.
"""

import sys

sys.path.insert(0, "/opt/trn_rl_repo")

import math

import ml_dtypes
import numpy as np

import concourse.bass as bass
from concourse import bacc
import concourse.mybir as mybir
import concourse.tile as tile
from concourse.bass_utils import run_bass_kernel_spmd

F32 = mybir.dt.float32
BF16 = mybir.dt.bfloat16
EXP = mybir.ActivationFunctionType.Exp
NP_BF16 = ml_dtypes.bfloat16

D_MODEL = 1024
N_HEADS = 16
D_HEAD = 64
BATCH = 2
SEQ = 2048
H_LOC = 4          # heads per core
CH = 512           # n-chunk (= strip) size
NCH = SEQ // CH    # 4 chunks
KT = 128           # k tile
NT = SEQ // KT     # 16 n-tiles


def build_program(reps=1):
    nc = bacc.Bacc()
    xT = nc.dram_tensor("xT", [D_MODEL, SEQ], BF16, kind="ExternalInput")
    wqk = nc.dram_tensor("wqk", [D_MODEL, 512], BF16, kind="ExternalInput")
    wv = nc.dram_tensor("wv", [D_MODEL, 256], BF16, kind="ExternalInput")
    wo = nc.dram_tensor("wo", [256, D_MODEL], BF16, kind="ExternalInput")
    cost = nc.dram_tensor("cost", [128, SEQ], F32, kind="ExternalInput")
    sint = nc.dram_tensor("sint", [128, SEQ], F32, kind="ExternalInput")
    biast = nc.dram_tensor("biast", [128, H_LOC * NT], F32, kind="ExternalInput")
    maskt = nc.dram_tensor("maskt", [128, 128], BF16, kind="ExternalInput")
    out = nc.dram_tensor("out", [SEQ, D_MODEL], BF16, kind="ExternalOutput")

    with tile.TileContext(nc) as tc:
        with (
            tc.tile_pool(name="consts", bufs=1) as consts,
            tc.tile_pool(name="persist", bufs=1) as persist,
            tc.tile_pool(name="rot", bufs=2) as rotp,
            tc.tile_pool(name="ptp", bufs=8) as ptp,
            tc.tile_pool(name="attnp", bufs=2) as attnp,
            tc.tile_pool(name="recp", bufs=4) as recp,
            tc.tile_pool(name="obp", bufs=4) as obp,
            tc.tile_pool(name="spps", bufs=4, space="PSUM") as spps,
            tc.tile_pool(name="prps", bufs=2, space="PSUM") as prps,
            tc.tile_pool(name="avps", bufs=2, space="PSUM") as avps,
        ):
            # ---- inputs staged in, ordered so the first projection's
            # operands (x chunk 0, wqk, wv) land first
            x_sb = persist.tile([128, 8, SEQ], BF16, tag="xsb")
            wqk_sb = consts.tile([128, 8, 512], BF16, tag="wqk")
            # first-half (k 0:4) operands of both x and wqk land first so the
            # k<4 accumulation steps of the first m-block start ~3us earlier
            nc.sync.dma_start(
                out=x_sb[:, 0:4, 0:CH],
                in_=xT[0:512, 0:CH].rearrange("(k p) m -> p k m", p=128),
            )
            nc.sync.dma_start(
                out=wqk_sb[:, 0:4, :],
                in_=wqk[0:512, :].rearrange("(k p) m -> p k m", p=128),
            )
            nc.sync.dma_start(
                out=x_sb[:, 4:8, 0:CH],
                in_=xT[512:1024, 0:CH].rearrange("(k p) m -> p k m", p=128),
            )
            nc.sync.dma_start(
                out=wqk_sb[:, 4:8, :],
                in_=wqk[512:1024, :].rearrange("(k p) m -> p k m", p=128),
            )
            wv_sb = consts.tile([128, 8, 256], BF16, tag="wv")
            nc.sync.dma_start(out=wv_sb, in_=wv.rearrange("(k p) m -> p k m", p=128))
            bias_sb = consts.tile([128, H_LOC * NT], F32, tag="bias")
            nc.sync.dma_start(out=bias_sb, in_=biast[:, :])
            mask_sb = consts.tile([128, 128], BF16, tag="mask")
            nc.sync.dma_start(out=mask_sb, in_=maskt[:, :])
            cos_sb = consts.tile([128, SEQ], F32, tag="cos")
            sin_sb = consts.tile([128, SEQ], F32, tag="sin")
            nc.sync.dma_start(out=cos_sb[:, 0:CH], in_=cost[:, 0:CH])
            nc.sync.dma_start(out=sin_sb[:, 0:CH], in_=sint[:, 0:CH])
            for c in range(1, NCH):
                nc.sync.dma_start(
                    out=x_sb[:, :, c * CH:(c + 1) * CH],
                    in_=xT[:, c * CH:(c + 1) * CH].rearrange(
                        "(k p) m -> p k m", p=128),
                )
                nc.sync.dma_start(out=cos_sb[:, c * CH:(c + 1) * CH],
                                  in_=cost[:, c * CH:(c + 1) * CH])
                nc.sync.dma_start(out=sin_sb[:, c * CH:(c + 1) * CH],
                                  in_=sint[:, c * CH:(c + 1) * CH])
            wo_sb = consts.tile([128, 2, D_MODEL], BF16, tag="wo")
            nc.sync.dma_start(out=wo_sb, in_=wo.rearrange("(k p) m -> p k m", p=128))

            # V in [n, dh] layout: [128, ntile, head, 128]; per head block,
            # cols 0:64 = V, cols 64:128 = ones (denominator-replication
            # trick). memset through an f32 view: two bf16 1.0s per f32 lane
            # (native bf16 memset patterns are unreliable on HW)
            # memset on the otherwise-idle Pool engine: at kernel start the
            # DVE queue is the critical path (rotations -> repack -> scores)
            v_sb = persist.tile([128, NT, H_LOC, 128], BF16, tag="vsb")
            ones2 = float(np.frombuffer(np.uint32(0x3F803F80).tobytes(), np.float32)[0])
            nc.gpsimd.memset(v_sb.bitcast(F32)[:, :, :, 32:64], ones2)

            # packed rotated Q/K, head-pair layout
            qb = [persist.tile([128, SEQ], BF16, tag=f"qb{j}", name=f"qb{j}") for j in range(2)]
            kb = [persist.tile([128, SEQ], BF16, tag=f"kb{j}", name=f"kb{j}") for j in range(2)]

            attn_tiles = {}  # (strip, pair) -> sbuf tile [128, 512] bf16

            def rotate(pe, po, dst, c0):
                # pe/po: psum [128, CH] stacked evens/odds for 4 heads
                # dst: [buf01, buf23]; writes rotated head-pair-packed layout
                t1 = rotp.tile([128, CH], F32, tag="t1")
                t2 = rotp.tile([128, CH], F32, tag="t2")
                top = rotp.tile([128, CH], BF16, tag="top")
                bot = rotp.tile([128, CH], BF16, tag="bot")
                cs = cos_sb[:, c0:c0 + CH]
                sn = sin_sb[:, c0:c0 + CH]
                nc.vector.tensor_mul(t1[:, :], pe[:, :], cs)
                nc.vector.tensor_mul(t2[:, :], po[:, :], sn)
                nc.vector.tensor_sub(top[:, :], t1[:, :], t2[:, :])
                nc.vector.tensor_mul(t1[:, :], pe[:, :], sn)
                nc.vector.tensor_mul(t2[:, :], po[:, :], cs)
                nc.vector.tensor_add(bot[:, :], t1[:, :], t2[:, :])
                # repack: head h (32-row group) -> buf h//2, rows 64*(h%2)+{0:32 top, 32:64 bot}
                for h in range(4):
                    b = dst[h // 2]
                    r0 = 64 * (h % 2)
                    nc.sync.dma_start(out=b[r0:r0 + 32, c0:c0 + CH], in_=top[32 * h:32 * h + 32, :])
                    nc.sync.dma_start(out=b[r0 + 32:r0 + 64, c0:c0 + CH], in_=bot[32 * h:32 * h + 32, :])

            def qk_units(c):
                # generator: yields after each ~1-2us PE unit so the driver
                # can interleave attention tiles between units. Rotations
                # reach the DVE queue right after each pair block -- the
                # next strip's scores wait on the repack.
                c0 = c * CH
                for mp in range(2):
                    pe = prps.tile([128, CH], F32, tag="pr", name="pe")
                    po = prps.tile([128, CH], F32, tag="pr", name="po")
                    for m2, dst in ((0, pe), (1, po)):
                        m = 2 * mp + m2
                        for k in range(8):
                            nc.tensor.matmul(
                                dst[:, :],
                                wqk_sb[:, k, m * 128:(m + 1) * 128],
                                x_sb[:, k, c0:c0 + CH],
                                start=(k == 0), stop=(k == 7),
                            )
                        yield
                    rotate(pe, po, qb if mp == 0 else kb, c0)

            def v_units(c):
                c0 = c * CH
                for it in range(4):
                    t = 4 * c + it
                    vp = prps.tile([128, 256], F32, tag="pr", name="vp")
                    for k in range(8):
                        nc.tensor.matmul(
                            vp[:, :],
                            x_sb[:, k, c0 + it * 128:c0 + (it + 1) * 128],
                            wv_sb[:, k, :],
                            start=(k == 0), stop=(k == 7),
                        )
                    nc.vector.tensor_copy(
                        out=v_sb[:, t, :, 0:64],
                        in_=vp[:, :].rearrange("p (h d) -> p h d", h=4),
                    )
                    yield

            def proj_units(c):
                yield from qk_units(c)
                yield from v_units(c)

            def attn_units(s):
                q0 = s * CH
                ntile_hi = 4 * s + 4
                for pair in range(2):
                    avs = [
                        avps.tile([128, CH], F32, tag="av", name=f"av_{s}_{pair}_{hl}")
                        for hl in range(2)
                    ]

                    def tile_geom(t):
                        r = t - 4 * s
                        qoff = 128 * r if r >= 0 else 0
                        return qoff, CH - qoff, r

                    def issue_scores(t):
                        qoff, w, r = tile_geom(t)
                        pts = []
                        for hl in range(2):
                            h = pair * 2 + hl
                            r0 = 64 * hl
                            sp = spps.tile([128, CH], F32, tag="sp", name=f"sp{hl}")
                            nc.tensor.matmul(
                                sp[:, 0:w],
                                kb[pair][r0:r0 + 64, t * KT:(t + 1) * KT],
                                qb[pair][r0:r0 + 64, q0 + qoff:q0 + CH],
                                start=True, stop=True,
                            )
                            pt = ptp.tile([128, CH], BF16, tag="pt", name=f"pt{hl}")
                            col = h * NT + t
                            nc.scalar.activation(
                                out=pt[:, 0:w], in_=sp[:, 0:w], func=EXP,
                                bias=bias_sb[:, col:col + 1], scale=1.0,
                            )
                            if r >= 0:
                                nc.gpsimd.tensor_mul(pt[:, 0:128], pt[:, 0:128], mask_sb[:, :])
                            pts.append(pt)
                        return pts

                    def issue_pv(t, pts, first=False, last=False):
                        qoff, w, r = tile_geom(t)
                        for hl in range(2):
                            h = pair * 2 + hl
                            nc.tensor.matmul(
                                avs[hl][:, qoff:CH],
                                v_sb[:, t, h, :],
                                pts[hl][:, 0:w],
                                start=first, stop=last,
                            )

                    # software pipeline: scores one tile ahead of PV
                    prev = None
                    for idx, t in enumerate(range(ntile_hi)):
                        cur = (t, issue_scores(t))
                        if prev is not None:
                            issue_pv(prev[0], prev[1], first=(idx == 1))
                        prev = cur
                        yield
                    issue_pv(prev[0], prev[1], last=True)

                    for hl in range(2):
                        r0 = 64 * hl
                        rec = recp.tile([64, CH], F32, tag="rec")
                        # tracked copy PSUM->SBUF first: custom-DVE ops don't
                        # get cross-engine dependency edges from Tile, so the
                        # copy provides the PE->DVE semaphore; the in-place
                        # approx then runs same-engine in-order
                        nc.vector.tensor_copy(out=rec[:, :], in_=avs[hl][64:128, :])
                        nc.vector.reciprocal_approx_fast(out=rec[:, :], in_=rec[:, :])
                        at = attn_tiles.get((s, pair))
                        if at is None:
                            at = attnp.tile([128, CH], BF16, tag=f"attn{pair}", name=f"attn_{s}_{pair}")
                            attn_tiles[(s, pair)] = at
                        nc.vector.tensor_mul(at[r0:r0 + 64, :], avs[hl][0:64, :], rec[:, :])
                    yield

            def oproj_units(s, copy_engine=None, fine=False):
                # copy_engine: the final strip runs its PSUM->SBUF copies on
                # the by-then-idle ACT engine so the busy DVE queue doesn't
                # delay the output tail. fine=True yields per half-tile for
                # denser interleaving.
                for it in range(4):
                    i = 4 * s + it
                    ob = obp.tile([128, 2, CH], BF16, tag="ob", name="ob")
                    for half in range(2):
                        op = spps.tile([128, CH], F32, tag="sp", name="op")
                        for ks in range(2):
                            nc.tensor.matmul(
                                op[:, :],
                                attn_tiles[(s, ks)][:, it * 128:(it + 1) * 128],
                                wo_sb[:, ks, half * CH:(half + 1) * CH],
                                start=(ks == 0), stop=(ks == 1),
                            )
                        if copy_engine == "scalar":
                            nc.scalar.copy(out=ob[:, half, :], in_=op[:, :])
                        else:
                            nc.vector.tensor_copy(out=ob[:, half, :], in_=op[:, :])
                        nc.sync.dma_start(
                            out=out[i * 128:(i + 1) * 128, half * CH:(half + 1) * CH],
                            in_=ob[:, half, :],
                        )
                        if fine:
                            yield
                    if not fine:
                        yield

            _END = object()

            def drain(g):
                for _ in g:
                    pass

            def interleave(fg, bg, ratio=1):
                # issue `ratio` foreground units, then 1 background unit,
                # until the foreground is exhausted; bg is NOT drained
                while True:
                    for _ in range(ratio):
                        if next(fg, _END) is _END:
                            return
                    next(bg, _END)

            def chain(*gens):
                for g in gens:
                    yield from g

            # attention strip s only needs chunks <= s. Strips interleave
            # at the tile level with the next chunk's projection and the
            # previous strip's O-projection: the PE always has independent
            # background work while the exp/normalize chains drain.
            for _rep in range(reps):
                attn_tiles.clear()
                drain(proj_units(0))
                bg = chain(proj_units(1))
                interleave(attn_units(0), bg)
                drain(bg)
                bg = chain(proj_units(2), oproj_units(0))
                interleave(attn_units(1), bg)
                drain(bg)
                bg = chain(qk_units(3), oproj_units(1))
                interleave(attn_units(2), bg, ratio=3)
                drain(bg)
                bg = chain(v_units(3), oproj_units(2, fine=True))
                interleave(attn_units(3), bg, ratio=3)
                drain(bg)
                drain(oproj_units(3, copy_engine="scalar"))

    return nc


def _sigmoid(v):
    return 1.0 / (1.0 + np.exp(-v.astype(np.float64)))


def build_inputs(x, Wqkv, Wo, log_xi, pi_gate_logit, e_gate_logit):
    x = np.asarray(x, np.float32)
    Wqkv = np.asarray(Wqkv, np.float32)
    Wo = np.asarray(Wo, np.float32)
    log_xi = np.asarray(log_xi, np.float32)
    pi_gate_logit = np.asarray(pi_gate_logit, np.float32)
    e_gate_logit = np.asarray(e_gate_logit, np.float32)

    pi_g = _sigmoid(pi_gate_logit)                      # (16,)
    c_h = (_sigmoid(e_gate_logit) / np.exp(log_xi.astype(np.float64)))  # (16,)

    Wq = Wqkv[0:1024].reshape(N_HEADS, D_HEAD, D_MODEL)
    Wk = Wqkv[1024:2048].reshape(N_HEADS, D_HEAD, D_MODEL)
    Wv = Wqkv[2048:3072].reshape(N_HEADS, D_HEAD, D_MODEL)

    f = np.arange(32)
    inv_freq = np.float64(math.pi) ** (1.0 - 2.0 * f / 64.0)            # (32,)
    pos = np.arange(SEQ, dtype=np.float64)

    mask01 = (np.arange(128)[:, None] <= np.arange(128)[None, :]).astype(NP_BF16)

    in_maps = []
    xTb = [np.ascontiguousarray(x[b].T).astype(NP_BF16) for b in range(BATCH)]
    for core in range(8):
        b, g = core // 4, core % 4
        hs = slice(4 * g, 4 * g + 4)
        qe = (Wq[hs, 0::2, :] * 0.125).reshape(128, D_MODEL)
        qo = (Wq[hs, 1::2, :] * 0.125).reshape(128, D_MODEL)
        ke = Wk[hs, 0::2, :].reshape(128, D_MODEL)
        ko = Wk[hs, 1::2, :].reshape(128, D_MODEL)
        wqk = np.ascontiguousarray(np.concatenate([qe, qo, ke, ko], 0).T).astype(NP_BF16)
        wv = np.ascontiguousarray(Wv[hs].reshape(256, D_MODEL).T).astype(NP_BF16)
        wo = np.ascontiguousarray(Wo[:, 256 * g:256 * (g + 1)].T).astype(NP_BF16)

        theta = pos[None, None, :] * inv_freq[None, :, None] * pi_g[4 * g:4 * g + 4, None, None]
        cost = np.cos(theta).reshape(128, SEQ).astype(np.float32)
        sint = np.sin(theta).reshape(128, SEQ).astype(np.float32)

        biast = np.empty((128, H_LOC * NT), np.float32)
        p = np.arange(128, dtype=np.float64)
        for hl in range(H_LOC):
            for t in range(NT):
                biast[:, hl * NT + t] = (c_h[4 * g + hl] * (128 * t + p)).astype(np.float32)

        in_maps.append({
            "xT": xTb[b], "wqk": wqk, "wv": wv, "wo": wo,
            "cost": cost, "sint": sint, "biast": biast, "maskt": mask01,
        })
    return in_maps


def kernel(x, Wqkv, Wo, log_xi, pi_gate_logit, e_gate_logit):
    in_maps = build_inputs(x, Wqkv, Wo, log_xi, pi_gate_logit, e_gate_logit)
    nc = build_program()
    nc.finalize()
    res = run_bass_kernel_spmd(nc, in_maps, list(range(8))).results
    out = np.zeros((BATCH, SEQ, D_MODEL), np.float32)
    for core in range(8):
        out[core // 4] += np.asarray(res[core]["out"]).astype(np.float32)
    return out


# revision 51
# speedup vs baseline: 1.0522x; 1.0182x over previous
"""EulerCE attention Trainium2 kernel (bf16, interleaved schedule).

Sharding: data-parallel over batch (2) x head-parallel over 4 head-groups
(16 heads / 4 per group) = 8 cores. Core c: batch c//4, heads 4*(c%4)..+4.

Per-core math (head group g, batch b):
  - All matmul operands in bf16 (fast weight loads); PSUM stays f32.
  - Full xT staged to SBUF up-front, DMA order chosen so the first
    projection's operands land first (PE starts ~6us in).
  - QKV projection with host-permuted weight rows so Q/K come out in
    "stacked evens/odds" layout ready for a full-128-partition RoPE-style
    rotation on DVE; V computed in [n, dh] orientation directly.
  - scores computed transposed: s^T[k, q] = K_h^T-slice . Q_h-slice,
    decay bias folded into the exp's per-partition bias (c_h * k is a
    per-partition value in this layout; the -c_h*q per-row term cancels in
    softmax), causal mask applied only on exact-diagonal 128x128 subtiles
    (Pool engine, SBUF-only).
  - softmax without max-subtraction (scores provably small for this data),
    denominator obtained by 64 ones-columns in the PV stationary operand
    which makes the PE replicate sum_k P across 64 partitions for free.
  - softmax normalization uses reciprocal_approx_fast after a tracked
    PSUM->SBUF copy (custom-DVE ops don't get cross-engine dep edges).
  - phases are split into ~1-2us issue units and interleaved: attention
    tiles run between projection/O-projection units so the PE always has
    independent work while exp/normalize chains drain, and stays dense
    enough that HAM keeps the PE at full clock.
  - PSUM (8 banks): score tiles 4x1 (two tiles/attention-tile, 2-deep
    lookahead), PV accumulators 4x1 (pairs fully double-buffered),
    projection pair-tiles 2x2-bank ring shared with the V-proj psum and
    the O-proj outputs.
  - O-projection consumes attn^T directly; per-core partial outputs (bf16)
    are summed on host across the 4 head-group cores of each batch.
"""

import sys

sys.path.insert(0, "/opt/trn_rl_repo")

import math

import ml_dtypes
import numpy as np

import concourse.bass as bass
from concourse import bacc
import concourse.mybir as mybir
import concourse.tile as tile
from concourse.bass_utils import run_bass_kernel_spmd

F32 = mybir.dt.float32
BF16 = mybir.dt.bfloat16
EXP = mybir.ActivationFunctionType.Exp
NP_BF16 = ml_dtypes.bfloat16

D_MODEL = 1024
N_HEADS = 16
D_HEAD = 64
BATCH = 2
SEQ = 2048
H_LOC = 4          # heads per core
CH = 512           # n-chunk (= strip) size
NCH = SEQ // CH    # 4 chunks
KT = 128           # k tile
NT = SEQ // KT     # 16 n-tiles


def build_program(reps=1):
    nc = bacc.Bacc()
    xT = nc.dram_tensor("xT", [D_MODEL, SEQ], BF16, kind="ExternalInput")
    wqk = nc.dram_tensor("wqk", [D_MODEL, 512], BF16, kind="ExternalInput")
    wv = nc.dram_tensor("wv", [D_MODEL, 256], BF16, kind="ExternalInput")
    wo = nc.dram_tensor("wo", [256, D_MODEL], BF16, kind="ExternalInput")
    cost = nc.dram_tensor("cost", [128, SEQ], F32, kind="ExternalInput")
    sint = nc.dram_tensor("sint", [128, SEQ], F32, kind="ExternalInput")
    biast = nc.dram_tensor("biast", [128, H_LOC * NT], F32, kind="ExternalInput")
    maskt = nc.dram_tensor("maskt", [128, 128], BF16, kind="ExternalInput")
    out = nc.dram_tensor("out", [SEQ, D_MODEL], BF16, kind="ExternalOutput")

    with tile.TileContext(nc) as tc:
        with (
            tc.tile_pool(name="consts", bufs=1) as consts,
            tc.tile_pool(name="persist", bufs=1) as persist,
            tc.tile_pool(name="rot", bufs=4) as rotp,
            tc.tile_pool(name="ptp", bufs=8) as ptp,
            tc.tile_pool(name="attnp", bufs=3) as attnp,
            tc.tile_pool(name="recp", bufs=4) as recp,
            tc.tile_pool(name="obp", bufs=4) as obp,
            tc.tile_pool(name="spps", bufs=4, space="PSUM") as spps,
            tc.tile_pool(name="prps", bufs=2, space="PSUM") as prps,
            tc.tile_pool(name="avps", bufs=2, space="PSUM") as avps,
        ):
            # ---- inputs staged in, ordered so the first projection's
            # operands (x chunk 0, wqk, wv) land first
            x_sb = persist.tile([128, 8, SEQ], BF16, tag="xsb")
            wqk_sb = consts.tile([128, 8, 512], BF16, tag="wqk")
            # first-half (k 0:4) operands of both x and wqk land first so the
            # k<4 accumulation steps of the first m-block start ~3us earlier
            nc.sync.dma_start(
                out=x_sb[:, 0:4, 0:CH],
                in_=xT[0:512, 0:CH].rearrange("(k p) m -> p k m", p=128),
            )
            nc.sync.dma_start(
                out=wqk_sb[:, 0:4, :],
                in_=wqk[0:512, :].rearrange("(k p) m -> p k m", p=128),
            )
            nc.sync.dma_start(
                out=x_sb[:, 4:8, 0:CH],
                in_=xT[512:1024, 0:CH].rearrange("(k p) m -> p k m", p=128),
            )
            nc.sync.dma_start(
                out=wqk_sb[:, 4:8, :],
                in_=wqk[512:1024, :].rearrange("(k p) m -> p k m", p=128),
            )
            wv_sb = consts.tile([128, 8, 256], BF16, tag="wv")
            nc.sync.dma_start(out=wv_sb, in_=wv.rearrange("(k p) m -> p k m", p=128))
            bias_sb = consts.tile([128, H_LOC * NT], F32, tag="bias")
            nc.sync.dma_start(out=bias_sb, in_=biast[:, :])
            mask_sb = consts.tile([128, 128], BF16, tag="mask")
            nc.sync.dma_start(out=mask_sb, in_=maskt[:, :])
            cos_sb = consts.tile([128, SEQ], F32, tag="cos")
            sin_sb = consts.tile([128, SEQ], F32, tag="sin")
            nc.sync.dma_start(out=cos_sb[:, 0:CH], in_=cost[:, 0:CH])
            nc.sync.dma_start(out=sin_sb[:, 0:CH], in_=sint[:, 0:CH])
            for c in range(1, NCH):
                nc.sync.dma_start(
                    out=x_sb[:, :, c * CH:(c + 1) * CH],
                    in_=xT[:, c * CH:(c + 1) * CH].rearrange(
                        "(k p) m -> p k m", p=128),
                )
                nc.sync.dma_start(out=cos_sb[:, c * CH:(c + 1) * CH],
                                  in_=cost[:, c * CH:(c + 1) * CH])
                nc.sync.dma_start(out=sin_sb[:, c * CH:(c + 1) * CH],
                                  in_=sint[:, c * CH:(c + 1) * CH])
            wo_sb = consts.tile([128, 2, D_MODEL], BF16, tag="wo")
            nc.sync.dma_start(out=wo_sb, in_=wo.rearrange("(k p) m -> p k m", p=128))

            # V in [n, dh] layout: [128, ntile, head, 128]; per head block,
            # cols 0:64 = V, cols 64:128 = ones (denominator-replication
            # trick). memset through an f32 view: two bf16 1.0s per f32 lane
            # (native bf16 memset patterns are unreliable on HW)
            # memset on the otherwise-idle Pool engine: at kernel start the
            # DVE queue is the critical path (rotations -> repack -> scores)
            v_sb = persist.tile([128, NT, H_LOC, 128], BF16, tag="vsb")
            ones2 = float(np.frombuffer(np.uint32(0x3F803F80).tobytes(), np.float32)[0])
            nc.gpsimd.memset(v_sb.bitcast(F32)[:, :, :, 32:64], ones2)

            # packed rotated Q/K, head-pair layout
            qb = [persist.tile([128, SEQ], BF16, tag=f"qb{j}", name=f"qb{j}") for j in range(2)]
            kb = [persist.tile([128, SEQ], BF16, tag=f"kb{j}", name=f"kb{j}") for j in range(2)]

            attn_tiles = {}  # (strip, pair) -> sbuf tile [128, 512] bf16

            def rotate(pe, po, dst, c0):
                # pe/po: psum [128, CH] stacked evens/odds for 4 heads
                # dst: [buf01, buf23]; writes rotated head-pair-packed layout
                t1 = rotp.tile([128, CH], F32, tag="t1")
                t2 = rotp.tile([128, CH], F32, tag="t2")
                top = rotp.tile([128, CH], BF16, tag="top")
                bot = rotp.tile([128, CH], BF16, tag="bot")
                cs = cos_sb[:, c0:c0 + CH]
                sn = sin_sb[:, c0:c0 + CH]
                nc.vector.tensor_mul(t1[:, :], pe[:, :], cs)
                nc.vector.tensor_mul(t2[:, :], po[:, :], sn)
                nc.vector.tensor_sub(top[:, :], t1[:, :], t2[:, :])
                nc.vector.tensor_mul(t1[:, :], pe[:, :], sn)
                nc.vector.tensor_mul(t2[:, :], po[:, :], cs)
                nc.vector.tensor_add(bot[:, :], t1[:, :], t2[:, :])
                # repack: head h (32-row group) -> buf h//2, rows 64*(h%2)+{0:32 top, 32:64 bot}
                for h in range(4):
                    b = dst[h // 2]
                    r0 = 64 * (h % 2)
                    nc.sync.dma_start(out=b[r0:r0 + 32, c0:c0 + CH], in_=top[32 * h:32 * h + 32, :])
                    nc.sync.dma_start(out=b[r0 + 32:r0 + 64, c0:c0 + CH], in_=bot[32 * h:32 * h + 32, :])

            def qk_units(c):
                # generator: yields after each ~1-2us PE unit so the driver
                # can interleave attention tiles between units. Rotations
                # reach the DVE queue right after each pair block -- the
                # next strip's scores wait on the repack.
                c0 = c * CH
                for mp in range(2):
                    pe = prps.tile([128, CH], F32, tag="pr", name="pe")
                    po = prps.tile([128, CH], F32, tag="pr", name="po")
                    for m2, dst in ((0, pe), (1, po)):
                        m = 2 * mp + m2
                        for k in range(8):
                            nc.tensor.matmul(
                                dst[:, :],
                                wqk_sb[:, k, m * 128:(m + 1) * 128],
                                x_sb[:, k, c0:c0 + CH],
                                start=(k == 0), stop=(k == 7),
                            )
                        yield
                    rotate(pe, po, qb if mp == 0 else kb, c0)

            def v_units(c):
                c0 = c * CH
                for it in range(4):
                    t = 4 * c + it
                    vp = prps.tile([128, 256], F32, tag="pr", name="vp")
                    for k in range(8):
                        nc.tensor.matmul(
                            vp[:, :],
                            x_sb[:, k, c0 + it * 128:c0 + (it + 1) * 128],
                            wv_sb[:, k, :],
                            start=(k == 0), stop=(k == 7),
                        )
                    nc.vector.tensor_copy(
                        out=v_sb[:, t, :, 0:64],
                        in_=vp[:, :].rearrange("p (h d) -> p h d", h=4),
                    )
                    yield

            def proj_units(c):
                yield from qk_units(c)
                yield from v_units(c)

            def attn_units(s):
                q0 = s * CH
                ntile_hi = 4 * s + 4
                for pair in range(2):
                    avs = [
                        avps.tile([128, CH], F32, tag="av", name=f"av_{s}_{pair}_{hl}")
                        for hl in range(2)
                    ]

                    def tile_geom(t):
                        r = t - 4 * s
                        qoff = 128 * r if r >= 0 else 0
                        return qoff, CH - qoff, r

                    def issue_scores(t):
                        qoff, w, r = tile_geom(t)
                        pts = []
                        for hl in range(2):
                            h = pair * 2 + hl
                            r0 = 64 * hl
                            sp = spps.tile([128, CH], F32, tag="sp", name=f"sp{hl}")
                            nc.tensor.matmul(
                                sp[:, 0:w],
                                kb[pair][r0:r0 + 64, t * KT:(t + 1) * KT],
                                qb[pair][r0:r0 + 64, q0 + qoff:q0 + CH],
                                start=True, stop=True,
                            )
                            pt = ptp.tile([128, CH], BF16, tag="pt", name=f"pt{hl}")
                            col = h * NT + t
                            nc.scalar.activation(
                                out=pt[:, 0:w], in_=sp[:, 0:w], func=EXP,
                                bias=bias_sb[:, col:col + 1], scale=1.0,
                            )
                            if r >= 0:
                                nc.gpsimd.tensor_mul(pt[:, 0:128], pt[:, 0:128], mask_sb[:, :])
                            pts.append(pt)
                        return pts

                    def issue_pv(t, pts, first=False, last=False):
                        qoff, w, r = tile_geom(t)
                        for hl in range(2):
                            h = pair * 2 + hl
                            nc.tensor.matmul(
                                avs[hl][:, qoff:CH],
                                v_sb[:, t, h, :],
                                pts[hl][:, 0:w],
                                start=first, stop=last,
                            )

                    # software pipeline: scores one tile ahead of PV
                    prev = None
                    for idx, t in enumerate(range(ntile_hi)):
                        cur = (t, issue_scores(t))
                        if prev is not None:
                            issue_pv(prev[0], prev[1], first=(idx == 1))
                        prev = cur
                        yield
                    issue_pv(prev[0], prev[1], last=True)

                    for hl in range(2):
                        r0 = 64 * hl
                        rec = recp.tile([64, CH], F32, tag="rec")
                        # tracked copy PSUM->SBUF first: custom-DVE ops don't
                        # get cross-engine dependency edges from Tile, so the
                        # copy provides the PE->DVE semaphore; the in-place
                        # approx then runs same-engine in-order
                        nc.vector.tensor_copy(out=rec[:, :], in_=avs[hl][64:128, :])
                        nc.vector.reciprocal_approx_fast(out=rec[:, :], in_=rec[:, :])
                        at = attn_tiles.get((s, pair))
                        if at is None:
                            at = attnp.tile([128, CH], BF16, tag=f"attn{pair}", name=f"attn_{s}_{pair}")
                            attn_tiles[(s, pair)] = at
                        nc.vector.tensor_mul(at[r0:r0 + 64, :], avs[hl][0:64, :], rec[:, :])
                    yield

            def oproj_units(s, copy_engine=None, fine=False):
                # copy_engine: the final strip runs its PSUM->SBUF copies on
                # the by-then-idle ACT engine so the busy DVE queue doesn't
                # delay the output tail. fine=True yields per half-tile for
                # denser interleaving.
                for it in range(4):
                    i = 4 * s + it
                    ob = obp.tile([128, 2, CH], BF16, tag="ob", name="ob")
                    for half in range(2):
                        op = spps.tile([128, CH], F32, tag="sp", name="op")
                        for ks in range(2):
                            nc.tensor.matmul(
                                op[:, :],
                                attn_tiles[(s, ks)][:, it * 128:(it + 1) * 128],
                                wo_sb[:, ks, half * CH:(half + 1) * CH],
                                start=(ks == 0), stop=(ks == 1),
                            )
                        if copy_engine == "scalar":
                            nc.scalar.copy(out=ob[:, half, :], in_=op[:, :])
                        else:
                            nc.vector.tensor_copy(out=ob[:, half, :], in_=op[:, :])
                        nc.sync.dma_start(
                            out=out[i * 128:(i + 1) * 128, half * CH:(half + 1) * CH],
                            in_=ob[:, half, :],
                        )
                        if fine:
                            yield
                    if not fine:
                        yield

            _END = object()

            def drain(g):
                for _ in g:
                    pass

            def interleave(fg, bg, ratio=1):
                # issue `ratio` foreground units, then 1 background unit,
                # until the foreground is exhausted; bg is NOT drained
                while True:
                    for _ in range(ratio):
                        if next(fg, _END) is _END:
                            return
                    next(bg, _END)

            def chain(*gens):
                for g in gens:
                    yield from g

            # attention strip s only needs chunks <= s. Strips interleave
            # at the tile level with the next chunk's projection and the
            # previous strip's O-projection: the PE always has independent
            # background work while the exp/normalize chains drain.
            for _rep in range(reps):
                attn_tiles.clear()
                drain(proj_units(0))
                bg = chain(proj_units(1))
                interleave(attn_units(0), bg)
                drain(bg)
                bg = chain(proj_units(2), oproj_units(0))
                interleave(attn_units(1), bg)
                drain(bg)
                bg = chain(qk_units(3), oproj_units(1))
                interleave(attn_units(2), bg, ratio=3)
                drain(bg)
                bg = chain(v_units(3), oproj_units(2, fine=True))
                interleave(attn_units(3), bg, ratio=3)
                drain(bg)
                drain(oproj_units(3, copy_engine="scalar"))

    return nc


def _sigmoid(v):
    return 1.0 / (1.0 + np.exp(-v.astype(np.float64)))


def build_inputs(x, Wqkv, Wo, log_xi, pi_gate_logit, e_gate_logit):
    x = np.asarray(x, np.float32)
    Wqkv = np.asarray(Wqkv, np.float32)
    Wo = np.asarray(Wo, np.float32)
    log_xi = np.asarray(log_xi, np.float32)
    pi_gate_logit = np.asarray(pi_gate_logit, np.float32)
    e_gate_logit = np.asarray(e_gate_logit, np.float32)

    pi_g = _sigmoid(pi_gate_logit)                      # (16,)
    c_h = (_sigmoid(e_gate_logit) / np.exp(log_xi.astype(np.float64)))  # (16,)

    Wq = Wqkv[0:1024].reshape(N_HEADS, D_HEAD, D_MODEL)
    Wk = Wqkv[1024:2048].reshape(N_HEADS, D_HEAD, D_MODEL)
    Wv = Wqkv[2048:3072].reshape(N_HEADS, D_HEAD, D_MODEL)

    f = np.arange(32)
    inv_freq = np.float64(math.pi) ** (1.0 - 2.0 * f / 64.0)            # (32,)
    pos = np.arange(SEQ, dtype=np.float64)

    mask01 = (np.arange(128)[:, None] <= np.arange(128)[None, :]).astype(NP_BF16)

    in_maps = []
    xTb = [np.ascontiguousarray(x[b].T).astype(NP_BF16) for b in range(BATCH)]
    for core in range(8):
        b, g = core // 4, core % 4
        hs = slice(4 * g, 4 * g + 4)
        qe = (Wq[hs, 0::2, :] * 0.125).reshape(128, D_MODEL)
        qo = (Wq[hs, 1::2, :] * 0.125).reshape(128, D_MODEL)
        ke = Wk[hs, 0::2, :].reshape(128, D_MODEL)
        ko = Wk[hs, 1::2, :].reshape(128, D_MODEL)
        wqk = np.ascontiguousarray(np.concatenate([qe, qo, ke, ko], 0).T).astype(NP_BF16)
        wv = np.ascontiguousarray(Wv[hs].reshape(256, D_MODEL).T).astype(NP_BF16)
        wo = np.ascontiguousarray(Wo[:, 256 * g:256 * (g + 1)].T).astype(NP_BF16)

        theta = pos[None, None, :] * inv_freq[None, :, None] * pi_g[4 * g:4 * g + 4, None, None]
        cost = np.cos(theta).reshape(128, SEQ).astype(np.float32)
        sint = np.sin(theta).reshape(128, SEQ).astype(np.float32)

        biast = np.empty((128, H_LOC * NT), np.float32)
        p = np.arange(128, dtype=np.float64)
        for hl in range(H_LOC):
            for t in range(NT):
                biast[:, hl * NT + t] = (c_h[4 * g + hl] * (128 * t + p)).astype(np.float32)

        in_maps.append({
            "xT": xTb[b], "wqk": wqk, "wv": wv, "wo": wo,
            "cost": cost, "sint": sint, "biast": biast, "maskt": mask01,
        })
    return in_maps


def kernel(x, Wqkv, Wo, log_xi, pi_gate_logit, e_gate_logit):
    in_maps = build_inputs(x, Wqkv, Wo, log_xi, pi_gate_logit, e_gate_logit)
    nc = build_program()
    nc.finalize()
    res = run_bass_kernel_spmd(nc, in_maps, list(range(8))).results
    out = np.zeros((BATCH, SEQ, D_MODEL), np.float32)
    for core in range(8):
        out[core // 4] += np.asarray(res[core]["out"]).astype(np.float32)
    return out
